# revision 19
# baseline (speedup 1.0000x reference)
"""Bilinear RGGB demosaic (Bayer -> RGB) on 8 Trainium2 NeuronCores.

Layout: batch image n -> core n. Per core, the [2048, 2048] mosaic is
processed in 8 bands of exactly 128 row-pairs; SBUF partition p of a
band holds the row pair (2p, 2p+1) concatenated in the free dim, so
every DRAM transfer is 16 KiB-contiguous per partition.

Vertical stencil taps are banded 128x128 matmuls (fp32r) on the tensor
engine; band-seam rows are fixed up with K=1 matmuls against the
neighbor band's input tile (accumulated into the same PSUM bank, so no
halo DMA and no overlap bands). The G channel is computed ENTIRELY on
the tensor engine: its horizontal taps are expressed as additional
accumulating matmuls whose moving operands are column-shifted APs of
the same input tile (PSUM accumulation = free adds). Su4/Sd4 are only
produced at the column parity their R/B consumers need, which also
makes every remaining DVE add contiguous.

Inputs are cast f32->bf16 inline by the GpSimd SWDGE input DMAs (the
cast rides the transfer; PE then runs full-rate bf16 matmuls), and
outputs are written bf16 (tolerance 2e-2; bf16 rounds at ~2e-3),
halving store traffic; the host widens back to f32. Output DMAs ride
the SP HWDGE ring (nc.sync) so neither compute engine's instruction
stream ever blocks on a store; per-band work is ordered Sd->R, Su->B,
Ge/Go->G so each channel's stores launch while later matmuls still
run. Elementwise work is balanced across DVE and ACT. At 8 bands the
kernel sustains ~345 GB/s of HBM traffic (~97% of the 358 GB/s
per-core cap) — it is memory-roofline bound.
"""

import sys

sys.path.insert(0, "/opt/trn_rl_repo")

import numpy as np

import concourse.bass as bass
import concourse.tile as tile
from concourse import mybir
from concourse.alu_op_type import AluOpType
from concourse.bass_utils import run_bass_kernel_spmd

F32 = mybir.dt.float32
BF16 = mybir.dt.bfloat16
N_CORES = 8
H = 2048
W = 2048
NBANDS = H // 256  # 8 bands of 128 row-pairs


def split_sync_waits(nc, max_waits=1):
    """This walrus build rejects instructions carrying more than
    `max_waits` sync-wait commands. Hoist excess waits onto same-engine
    NoOps inserted immediately before the over-subscribed instruction
    (waiting earlier on the same queue is semantically conservative)."""
    for fn in nc.m.functions:
        for bb in fn.blocks:
            insts = bb.instructions
            i = 0
            while i < len(insts):
                inst = insts[i]
                si = inst.sync_info
                waits = list(si.on_wait) if si and si.on_wait else []
                if len(waits) > max_waits:
                    si.on_wait = waits[:max_waits]
                    excess = waits[max_waits:]
                    for j in range(0, len(excess), max_waits):
                        nop = mybir.InstNoOp(
                            name=nc.get_next_instruction_name(), ins=[], outs=[]
                        )
                        nop.engine = inst.engine
                        nop.sync_info = mybir.SyncInfo(
                            on_wait=excess[j : j + max_waits], on_update=[]
                        )
                        nc.register_instruction(nop)
                        insts.insert(i, nop)
                        i += 1
                i += 1


def const_arrays():
    # cmm[:, 0:128]   m1 : Su[p] = 0.25*(O[p-1] + O[p])
    # cmm[:, 128:256] m2 : Sd[p] = 0.25*(E[p] + E[p+1])
    # cmm[:, 256:384] qI : 0.25 * I (horizontal quarter taps)
    import ml_dtypes

    m1 = 0.25 * (np.eye(128, dtype=np.float32) + np.eye(128, k=1, dtype=np.float32))
    m2 = 0.25 * (np.eye(128, dtype=np.float32) + np.eye(128, k=-1, dtype=np.float32))
    qI = 0.25 * np.eye(128, dtype=np.float32)
    cmm = np.concatenate([m1, m2, qI], axis=1).astype(ml_dtypes.bfloat16)  # [128, 384]
    # cfx[0, 0:128]   fu : row vector, 0.25 into partition 0   (+= 0.25*O_prev)
    # cfx[0, 128:256] fd : row vector, 0.25 into partition 127 (+= 0.25*E_next)
    cfx = np.zeros((1, 256), dtype=np.float32)
    cfx[0, 0] = 0.25
    cfx[0, 128 + 127] = 0.25
    return cmm, cfx.astype(ml_dtypes.bfloat16)


def build_program(npairs=H // 2, w=W):
    nc = bass.Bass("TRN2", target_bir_lowering=False, debug=False)
    x = nc.dram_tensor("x", [npairs, 2 * w], F32, kind="ExternalInput").ap()
    cmm = nc.dram_tensor("cmm", [128, 384], BF16, kind="ExternalInput").ap()
    cfx = nc.dram_tensor("cfx", [1, 256], BF16, kind="ExternalInput").ap()
    out = nc.dram_tensor("out", [3, npairs, 2 * w], BF16, kind="ExternalOutput").ap()

    hw = w // 2  # 1024: outputs of one parity
    qw = w // 4  # 512: one PSUM bank / one half-band of one parity
    nb = npairs // 128
    assert nb * 128 == npairs

    with tile.TileContext(nc) as tc:
        with (
            tc.tile_pool(name="consts", bufs=1) as cpool,
            tc.tile_pool(name="inp", bufs=5) as ipool,
            tc.tile_pool(name="psum", bufs=1, space="PSUM") as ppool,
            tc.tile_pool(name="mids", bufs=2) as mpool,
            tc.tile_pool(name="tmps", bufs=2) as tpool,
            tc.tile_pool(name="outs", bufs=4) as opool,
        ):
            cM = cpool.tile([128, 384], BF16)
            nc.sync.dma_start(cM[:], cmm[:])
            cF = cpool.tile([1, 256], BF16)
            nc.sync.dma_start(cF[:], cfx[:])

            mm = lambda ap: ap
            m1 = mm(cM[:, 0:128])
            m2 = mm(cM[:, 128:256])
            qI = mm(cM[:, 256:384])
            fu = mm(cF[:, 0:128])
            fd = mm(cF[:, 128:256])

            INs = {}

            def get_in(k):
                # two DMAs: E half lands first so Sd/R work starts earlier
                if k not in INs:
                    t = ipool.tile([128, 2 * w], BF16, tag="in")
                    r = slice(128 * k, 128 * k + 128)
                    nc.gpsimd.dma_start(t[:, 0:w], x[r, 0:w])
                    nc.gpsimd.dma_start(t[:, w : 2 * w], x[r, w : 2 * w])
                    INs[k] = t
                return INs[k]

            for k in range(nb):
                IN = get_in(k)
                INn = get_in(k + 1) if k + 1 < nb else None
                for kk in (k + 2, k + 3):
                    if kk < nb:
                        get_in(kk)  # deeper prefetch keeps HBM reads ahead

                E = IN[:, 0:w]
                O = IN[:, w : 2 * w]
                # halo rows, single partition each. The next band's first E
                # row is partition 0 of its input tile (a legal matmul base
                # partition); the previous band's last O row would sit at
                # partition 127, so re-fetch those 8 KiB into partition 0.
                Op = None
                if k > 0:
                    OpH = tpool.tile([1, w], BF16, tag="oph")
                    nc.gpsimd.dma_start(OpH[:], x[128 * k - 1 : 128 * k, w : 2 * w])
                    Op = OpH[:]
                En = INn[0:1, 0:w] if INn is not None else None

                # PSUM, per half h (columns [1024h, 1024h+1024)):
                #   SuP[h] [128,512]: Su4 at odd cols of the half
                #   SdP[h] [128,512]: Su4 at even cols
                #   GeP[h] [128,512]: G at (even row, even col)
                #   GoP[h] [128,512]: G at (odd row, odd col)
                SuP0 = ppool.tile([128, qw], F32, tag="su0")
                SuP1 = ppool.tile([128, qw], F32, tag="su1")
                SdP0 = ppool.tile([128, qw], F32, tag="sd0")
                SdP1 = ppool.tile([128, qw], F32, tag="sd1")
                GeP0 = ppool.tile([128, qw], F32, tag="ge0")
                GeP1 = ppool.tile([128, qw], F32, tag="ge1")
                GoP0 = ppool.tile([128, qw], F32, tag="go0")
                GoP1 = ppool.tile([128, qw], F32, tag="go1")
                SuP = [SuP0, SuP1]
                SdP = [SdP0, SdP1]
                GeP = [GeP0, GeP1]
                GoP = [GoP0, GoP1]

                MM = nc.tensor.matmul
                rows = slice(128 * k, 128 * k + 128)

                # ---- Sd matmuls (need only the E half of the input) ----
                for h in range(2):
                    c = hw * h
                    MM(SdP[h][:], m2, mm(E[:, c : c + hw : 2]),
                       start=True, stop=(INn is None))
                for h in range(2):
                    c = hw * h
                    if INn is not None:
                        MM(SdP[h][:], fd, mm(En[:, c : c + hw : 2]),
                           start=False, stop=True, skip_group_check=True)

                # Sd4e[j] = Sd4[2j] (contiguous, f32)
                Sd4e = mpool.tile([128, hw], F32, tag="sd4e")
                for h in range(2):
                    nc.scalar.copy(Sd4e[:, h * qw : (h + 1) * qw], SdP[h][:])

                Rt = opool.tile([128, 2 * w], BF16, tag="r")
                Gt = opool.tile([128, 2 * w], BF16, tag="g")
                Bt = opool.tile([128, 2 * w], BF16, tag="b")

                # ---- R channel (E + Sd4e only) ----
                # (e,e): passthrough E even cols
                nc.vector.tensor_copy(Rt[:, 0:w:2], E[:, 0:w:2])
                # (e,o): 0.5*(E[x-1] + E[x+1]); te padded to hw, col w-1 via pad
                te = tpool.tile([128, hw], F32, tag="te")
                nc.vector.tensor_add(te[:, 0 : hw - 1], E[:, 0 : w - 2 : 2], E[:, 2:w:2])
                nc.vector.tensor_copy(te[:, hw - 1 : hw], E[:, w - 2 : w - 1])
                nc.vector.tensor_scalar_mul(Rt[:, 1:w:2], te[:], 0.5)
                # (o,e): 2*Sd4 at even cols
                nc.scalar.mul(Rt[:, w : 2 * w : 2], Sd4e[:], 2.0)
                # (o,o): Sd4[x-1] + Sd4[x+1] = Sd4e[j] + Sd4e[j+1]; last col copy
                nc.vector.tensor_add(
                    Rt[:, w + 1 : 2 * w - 2 : 2], Sd4e[:, 0 : hw - 1], Sd4e[:, 1:hw]
                )
                nc.vector.tensor_copy(
                    Rt[:, 2 * w - 1 : 2 * w], Sd4e[:, hw - 1 : hw]
                )
                nc.sync.dma_start(out[0, rows, 0:w], Rt[:, 0:w])
                nc.sync.dma_start(out[0, rows, w : 2 * w], Rt[:, w : 2 * w])

                # ---- Su matmuls (O half) ----
                for h in range(2):
                    c = hw * h
                    MM(SuP[h][:], m1, mm(O[:, c + 1 : c + hw : 2]),
                       start=True, stop=(Op is None))
                for h in range(2):
                    c = hw * h
                    if Op is not None:
                        MM(SuP[h][:], fu, mm(Op[:, c + 1 : c + hw : 2]),
                           start=False, stop=True, skip_group_check=True)

                # Su4o[j] = Su4[2j+1] (contiguous, f32)
                Su4o = mpool.tile([128, hw], F32, tag="su4o")
                for h in range(2):
                    nc.scalar.copy(Su4o[:, h * qw : (h + 1) * qw], SuP[h][:])

                # ---- B channel (O + Su4o only) ----
                # (e,e): Su4[x-1] + Su4[x+1] = Su4o[j-1] + Su4o[j]; col 0 copy
                nc.vector.tensor_add(
                    Bt[:, 2 : w - 1 : 2], Su4o[:, 0 : hw - 1], Su4o[:, 1:hw]
                )
                nc.vector.tensor_copy(Bt[:, 0:1], Su4o[:, 0:1])
                # (e,o): 2*Su4 at odd cols
                nc.vector.tensor_scalar_mul(Bt[:, 1:w:2], Su4o[:], 2.0)
                # (o,e): 0.5*(O[x-1] + O[x+1]); col 0 via tb pad
                tb = tpool.tile([128, hw], F32, tag="tb")
                nc.vector.tensor_add(tb[:, 1:hw], O[:, 1 : w - 2 : 2], O[:, 3:w:2])
                nc.vector.tensor_copy(tb[:, 0:1], O[:, 1:2])
                nc.scalar.mul(Bt[:, w : 2 * w : 2], tb[:], 0.5)
                # (o,o): passthrough O odd cols
                nc.vector.tensor_copy(Bt[:, w + 1 : 2 * w : 2], O[:, 1:w:2])
                nc.sync.dma_start(out[2, rows, 0:w], Bt[:, 0:w])
                nc.sync.dma_start(out[2, rows, w : 2 * w], Bt[:, w : 2 * w])

                # ---- Ge matmuls: G(e,e) = Su4@even + 0.25*(E[x-1]+E[x+1]) ----
                for h in range(2):
                    c = hw * h
                    MM(GeP[h][:], m1, mm(O[:, c : c + hw : 2]),
                       start=True, stop=False)
                for h in range(2):
                    c = hw * h
                    if c == 0:
                        MM(GeP[h][:, 1:qw], qI, mm(E[:, 1 : hw - 1 : 2]),
                           start=False, stop=False, skip_group_check=True)
                    else:
                        MM(GeP[h][:], qI, mm(E[:, c - 1 : c + hw - 1 : 2]),
                           start=False, stop=False, skip_group_check=True)
                    MM(GeP[h][:], qI, mm(E[:, c + 1 : min(c + hw + 1, w) : 2]),
                       start=False, stop=(Op is None), skip_group_check=True)
                for h in range(2):
                    c = hw * h
                    if Op is not None:
                        MM(GeP[h][:], fu, mm(Op[:, c : c + hw : 2]),
                           start=False, stop=True, skip_group_check=True)

                # G even rows: (e,e) from PSUM + (e,o) passthrough E odd cols
                for h in range(2):
                    nc.scalar.copy(Gt[:, 2 * h * qw : 2 * (h + 1) * qw : 2], GeP[h][:])
                nc.vector.tensor_copy(Gt[:, 1:w:2], E[:, 1:w:2])
                nc.sync.dma_start(out[1, rows, 0:w], Gt[:, 0:w])

                # ---- Go matmuls: G(o,o) = Sd4@odd + 0.25*(O[x-1]+O[x+1]) ----
                for h in range(2):
                    c = hw * h
                    MM(GoP[h][:], m2, mm(E[:, c + 1 : c + hw : 2]),
                       start=True, stop=False)
                for h in range(2):
                    c = hw * h
                    MM(GoP[h][:], qI, mm(O[:, c : c + hw : 2]),
                       start=False, stop=False, skip_group_check=True)
                    if c + hw == w:
                        MM(GoP[h][:, 0 : qw - 1], qI, mm(O[:, c + 2 : c + hw : 2]),
                           start=False, stop=(INn is None), skip_group_check=True)
                    else:
                        MM(GoP[h][:], qI, mm(O[:, c + 2 : c + hw + 2 : 2]),
                           start=False, stop=(INn is None), skip_group_check=True)
                for h in range(2):
                    c = hw * h
                    if INn is not None:
                        MM(GoP[h][:], fd, mm(En[:, c + 1 : c + hw : 2]),
                           start=False, stop=True, skip_group_check=True)

                # G odd rows: (o,o) from PSUM + (o,e) passthrough O even cols
                for h in range(2):
                    nc.scalar.copy(
                        Gt[:, w + 2 * h * qw + 1 : w + 2 * (h + 1) * qw : 2], GoP[h][:]
                    )
                nc.vector.tensor_copy(Gt[:, w : 2 * w : 2], O[:, 0:w:2])
                nc.sync.dma_start(out[1, rows, w : 2 * w], Gt[:, w : 2 * w])

                if k - 1 in INs:
                    del INs[k - 1]

    split_sync_waits(nc)
    return nc


_CACHE = {}


def _get_program(npairs, w):
    key = (npairs, w)
    if key not in _CACHE:
        _CACHE[key] = build_program(npairs, w)
    return _CACHE[key]


def kernel(x: np.ndarray) -> np.ndarray:
    n, _, h, w = x.shape
    assert (n, h, w) == (N_CORES, H, W), x.shape
    nc = _get_program(H // 2, W)
    cmm, cfx = const_arrays()
    in_maps = []
    for i in range(N_CORES):
        img = np.ascontiguousarray(x[i, 0], dtype=np.float32).reshape(H // 2, 2 * W)
        in_maps.append({"x": img, "cmm": cmm, "cfx": cfx})
    res = run_bass_kernel_spmd(nc, in_maps, core_ids=list(range(N_CORES)))
    outs = [
        np.asarray(res.results[i]["out"]).astype(np.float32).reshape(3, H, W)[None]
        for i in range(N_CORES)
    ]
    return np.concatenate(outs, axis=0)
